# revision 19
# baseline (speedup 1.0000x reference)
import sys, os
sys.path.insert(0, '/opt/trn_rl_repo')
import numpy as np
import concourse.bass as bass
import concourse.mybir as mybir
import concourse.tile as tile
import concourse.bacc as bacc
from concourse.bass_utils import run_bass_kernel_spmd
from concourse.masks import make_identity
from collections import Counter

NC = 8
TRACE = False
LAST_EXEC_NS = []
LAST_RESULTS = []
LAST_WALL_S = []
_PROG_CACHE = {}
NUM_GRAPHS = 256
GPC = NUM_GRAPHS // NC  # graphs per core
P = 128


# ---------------------------------------------------------------- host helpers

def _bin_layout(widths_per_node, common_bins):
    """Given each node's slot-width and a common {width: padded_count} spec,
    return (order, grid) where grid maps (bin, node-idx-in-bin) -> (p, col).

    Returns:
      bins: list of (w, cnt, col0, rows) in ascending w order
      node_pos: for each node (in input order): (p, col0_of_its_row, w) or None
    """
    bins = []
    col = 0
    for w in sorted(common_bins):
        cnt = common_bins[w]
        rows = (cnt + P - 1) // P
        bins.append((w, cnt, col, rows))
        col += rows * w
    return bins, col


def _build_core_l1(core, x, src_g, dst_l, deg, n0, V_c, common_bins):
    """Build L1 staged arrays for one core.

    Slot grid: bins by W = in-deg+1 (incl self slot). Node i of bin w sits at
    partition i%128, its w slots at cols [col0 + (i//128)*w, +w).
    Flat layout of a [128, COLS] grid: index p*COLS + col.
    """
    bins, COLS = _bin_layout(None, common_bins)
    order = np.argsort(deg[n0:n0 + V_c], kind='stable')  # local nodes by W asc
    W_local = deg[n0:n0 + V_c][order]

    # per-edge: group by dst
    sort_e = np.argsort(dst_l, kind='stable')
    src_sorted = src_g[sort_e]
    # row_ptr over local nodes (natural local order)
    row_ptr = np.searchsorted(dst_l[sort_e], np.arange(V_c + 1))

    x_slot = np.zeros((P * COLS, 4), np.float32)
    deg_slot = np.ones((P * COLS,), np.float32)   # pad: deg=1 -> dinv=1, x=0
    NR = sum(r for (_, _, _, r) in bins)
    # node grid (for m readback): binned node (p, nodecol) -> local natural id
    node_map = -np.ones((P, NR), np.int64)

    ptr = 0  # index into order
    nodecol = 0
    for (w, cnt, col0, rows) in bins:
        nb = int(np.searchsorted(W_local, w, side='right') - ptr)
        assert nb <= cnt
        nodes = order[ptr:ptr + nb]              # local ids of this bin's nodes
        ptr += nb
        if nb == 0:
            nodecol += rows
            continue
        i = np.arange(nb)
        p = i % P
        r = i // P
        flat = p * COLS + col0 + r * w           # [nb]
        base = row_ptr[nodes]                    # [nb] first edge of each node
        for j in range(w - 1):                   # vectorized over nodes
            s = src_sorted[base + j]
            x_slot[flat + j] = x[s]
            deg_slot[flat + j] = deg[s]
        x_slot[flat + w - 1] = x[n0 + nodes]
        deg_slot[flat + w - 1] = deg[n0 + nodes]
        node_map[p, nodecol + r] = nodes
        nodecol += rows
    return x_slot, deg_slot, node_map, bins, COLS, NR


def _build_core_l2(core, src_g, dst_l, deg, batch_l, n0, V_c,
                   common_bins2, N):
    """L2 staging: src-major out-deg bins (self slots included).

    Returns m_src [P*NR2, 2], graphid [P*COLS2] f32, degdst [P*COLS2] f32.
    """
    bins2, COLS2 = _bin_layout(None, common_bins2)
    NR2 = sum(r for (_, _, _, r) in bins2)

    # out-counts per global node within this core's edge set (+1 for local self)
    w2 = np.bincount(src_g, minlength=N).astype(np.int64)
    w2[n0:n0 + V_c] += 1
    nodes = np.nonzero(w2)[0]
    order = nodes[np.argsort(w2[nodes], kind='stable')]
    w2_sorted = w2[order]

    # edges grouped by src
    sort_e = np.argsort(src_g, kind='stable')
    dst_sorted = dst_l[sort_e]
    eptr = np.searchsorted(src_g[sort_e], np.arange(N + 1))

    node_grid = -np.ones((P * NR2,), np.int64)
    graphid = np.zeros((P * COLS2,), np.float32)
    degdst = np.ones((P * COLS2,), np.float32)

    ptr = 0
    nodecol = 0
    for (w, cnt, col0, rows) in bins2:
        nb = int(np.searchsorted(w2_sorted, w, side='right') - ptr)
        assert nb <= cnt, (w, nb, cnt)
        nodes = order[ptr:ptr + nb]
        ptr += nb
        if nb == 0:
            nodecol += rows
            continue
        i = np.arange(nb)
        p = i % P
        r = i // P
        flat = p * COLS2 + col0 + r * w
        node_grid[p * NR2 + nodecol + r] = nodes
        base = eptr[nodes]
        is_local = (nodes >= n0) & (nodes < n0 + V_c)
        # local nodes have w-1 real edges + 1 self slot; remote have w edges
        nedge = np.where(is_local, w - 1, w)
        for j in range(w):
            sel = j < nedge
            if sel.any():
                ds = dst_sorted[base[sel] + j]
                graphid[flat[sel] + j] = batch_l[ds]
                degdst[flat[sel] + j] = deg[n0 + ds]
        # self slots for local nodes at position w-1
        if is_local.any():
            vloc = nodes[is_local] - n0
            graphid[flat[is_local] + w - 1] = batch_l[vloc]
            degdst[flat[is_local] + w - 1] = deg[n0 + vloc]
        nodecol += rows
    return node_grid, graphid, degdst, bins2, COLS2, NR2


# ---------------------------------------------------------------- device progs

def _build_l1_program(bins, COLS, NR):
    f32 = mybir.dt.float32
    nc = bacc.Bacc("TRN2", target_bir_lowering=False, debug=False,
                   num_devices=NC)
    x_in = nc.dram_tensor("x_slot", [P * COLS, 4], f32, kind="ExternalInput")
    d_in = nc.dram_tensor("deg_slot", [P * COLS], f32, kind="ExternalInput")
    W1_in = nc.dram_tensor("W1", [4, 16], f32, kind="ExternalInput")
    b1_in = nc.dram_tensor("b1", [16], f32, kind="ExternalInput")
    W2_in = nc.dram_tensor("W2", [16, 2], f32, kind="ExternalInput")
    Wl_in = nc.dram_tensor("Wl", [2, 2], f32, kind="ExternalInput")
    m_out = nc.dram_tensor("m", [P * NR, 2], f32, kind="ExternalOutput")

    NRp = ((NR + 3) // 4) * 4  # pad for 4-col transposes (unused here; per-col)
    with tile.TileContext(nc) as tc:
        with (
            tc.tile_pool(name="const", bufs=1) as cpool,
            tc.tile_pool(name="sb", bufs=2) as sb,
            tc.tile_pool(name="work", bufs=3) as wk,
            tc.tile_pool(name="ps", bufs=2, space="PSUM") as ps,
        ):
            # --- constants
            W1t = cpool.tile([4, 16], f32, tag="w1")
            nc.sync.dma_start(W1t[:], W1_in.ap())
            b1t = cpool.tile([16, 1], f32, tag="b1")
            nc.sync.dma_start(b1t[:], b1_in.ap()[:, None])
            # W2' = W2 @ Wl : lhsT = W2^T [2,16] via strided DMA, rhs = Wl
            W2T = cpool.tile([2, 16], f32, tag="w2t")
            nc.sync.dma_start(W2T[:], W2_in.ap().rearrange("a b -> b a"))
            Wlt = cpool.tile([2, 2], f32, tag="wl")
            nc.sync.dma_start(Wlt[:], Wl_in.ap())
            W2p_ps = ps.tile([16, 2], f32, tag="w2p")
            nc.tensor.matmul(W2p_ps[:], lhsT=W2T[:], rhs=Wlt[:],
                             start=True, stop=True)
            W2p = cpool.tile([16, 2], f32, tag="w2p_sb")
            nc.vector.tensor_copy(W2p[:], W2p_ps[:])

            # --- stream slots: g = rsqrt(deg) * x, then per-bin reduce
            a1 = cpool.tile([P, NR, 4], f32, tag="a1")
            dinvb = cpool.tile([P, NR], f32, tag="dinvb")
            nodecol = 0
            for (w, cnt, col0, rows) in bins:
                ncols = rows * w
                xs = wk.tile([P, ncols, 4], f32, tag="xs")
                nc.sync.dma_start(
                    xs[:], x_in.ap().rearrange("(p c) d -> p c d", p=P)
                    [:, col0:col0 + ncols, :])
                dg = wk.tile([P, ncols], f32, tag="dg")
                nc.sync.dma_start(
                    dg[:], d_in.ap().rearrange("(p c) -> p c", p=P)
                    [:, col0:col0 + ncols])
                di = wk.tile([P, ncols], f32, tag="di")
                nc.vector.reciprocal(di[:], dg[:])
                nc.scalar.sqrt(di[:], di[:])
                g = wk.tile([P, ncols, 4], f32, tag="g")
                nc.vector.tensor_tensor(
                    out=g[:], in0=xs[:],
                    in1=di[:][:, :, None].to_broadcast([P, ncols, 4]),
                    op=mybir.AluOpType.mult)
                # reduce over w (strided innermost): view [P, rows, w, 4]
                gv = g[:].rearrange("p (r w) d -> p r w d", w=w)
                gvt = bass.AP(gv.tensor, gv.offset,
                              [gv.ap[0], gv.ap[1], gv.ap[3], gv.ap[2]])
                nc.vector.tensor_reduce(
                    out=a1[:, nodecol:nodecol + rows, :], in_=gvt,
                    axis=mybir.AxisListType.X, op=mybir.AluOpType.add)
                nc.vector.memset(dinvb[:, nodecol:nodecol + rows],
                                 float(w) ** -0.5)
                nodecol += rows

            # --- a1' = dinv * a1
            a1p = cpool.tile([P, NR, 4], f32, tag="a1p")
            nc.vector.tensor_tensor(
                out=a1p[:], in0=a1[:],
                in1=dinvb[:][:, :, None].to_broadcast([P, NR, 4]),
                op=mybir.AluOpType.mult)

            # --- per node-column: m = dinv * relu(a1' @ W1 + b1) @ W2'
            mt = cpool.tile([P, NR, 2], f32, tag="mt")
            ident = cpool.tile([P, P], f32, tag="ident")
            make_identity(nc, ident[:])
            for c in range(NR):
                a1T_ps = ps.tile([4, P], f32, tag="a1T")
                nc.tensor.transpose(out=a1T_ps[:], in_=a1p[:, c, :],
                                    identity=ident[:])
                a1T = wk.tile([4, P], f32, tag="a1T_sb")
                nc.vector.tensor_copy(a1T[:], a1T_ps[:])
                h1_ps = ps.tile([16, P], f32, tag="h1")
                nc.tensor.matmul(h1_ps[:], lhsT=W1t[:], rhs=a1T[:],
                                 start=True, stop=True)
                h1 = wk.tile([16, P], f32, tag="h1_sb")
                nc.scalar.activation(h1[:], h1_ps[:],
                                     mybir.ActivationFunctionType.Relu,
                                     bias=b1t[:, 0:1], scale=1.0)
                m_ps = ps.tile([P, 2], f32, tag="mcol")
                nc.tensor.matmul(m_ps[:], lhsT=h1[:], rhs=W2p[:],
                                 start=True, stop=True)
                nc.vector.tensor_tensor(
                    out=mt[:, c, :], in0=m_ps[:],
                    in1=dinvb[:, c:c + 1].to_broadcast([P, 2]),
                    op=mybir.AluOpType.mult)
            nc.sync.dma_start(m_out.ap().rearrange("(p c) d -> p c d", p=P),
                              mt[:])
    nc.compile()
    return nc


def _build_l2_program(bins2, COLS2, NR2):
    f32 = mybir.dt.float32
    nc = bacc.Bacc("TRN2", target_bir_lowering=False, debug=False,
                   num_devices=NC)
    m_in = nc.dram_tensor("m_src", [P * NR2, 2], f32, kind="ExternalInput")
    g_in = nc.dram_tensor("graphid", [P * COLS2], f32, kind="ExternalInput")
    dd_in = nc.dram_tensor("degdst", [P * COLS2], f32, kind="ExternalInput")
    cnt_in = nc.dram_tensor("cnts", [GPC], f32, kind="ExternalInput")
    iota_in = nc.dram_tensor("iota32", [P, GPC], f32, kind="ExternalInput")
    W2_in = nc.dram_tensor("W2", [16, 2], f32, kind="ExternalInput")
    Wl_in = nc.dram_tensor("Wl", [2, 2], f32, kind="ExternalInput")
    b2_in = nc.dram_tensor("b2", [2], f32, kind="ExternalInput")
    bl_in = nc.dram_tensor("bl", [2], f32, kind="ExternalInput")
    out = nc.dram_tensor("out", [2, GPC], f32, kind="ExternalOutput")

    with tile.TileContext(nc) as tc:
        with (
            tc.tile_pool(name="const", bufs=1) as cpool,
            tc.tile_pool(name="wk", bufs=4) as wk,
            tc.tile_pool(name="ps", bufs=2, space="PSUM") as ps,
        ):
            iota = cpool.tile([P, GPC], f32, tag="iota")
            nc.sync.dma_start(iota[:], iota_in.ap())
            mg = cpool.tile([P, NR2, 2], f32, tag="mg")
            nc.sync.dma_start(
                mg[:], m_in.ap().rearrange("(p c) d -> p c d", p=P))

            acc = ps.tile([2, GPC], f32, tag="acc")
            nc.vector.memset(acc[:], 0.0)

            total_cols = sum(r * w for (w, _, _, r) in bins2)
            done_cols = 0
            nodecol = 0
            for (w, cnt, col0, rows) in bins2:
                ncols = rows * w
                # expand m by w
                v = wk.tile([P, ncols, 2], f32, tag="v")
                src = mg[:, nodecol:nodecol + rows, :]
                src_b = bass.AP(src.tensor, src.offset,
                                [src.ap[0], src.ap[1], [0, w], src.ap[2]])
                nc.vector.tensor_copy(
                    v[:].rearrange("p (r w) d -> p r w d", w=w), src_b)
                # dinv[dst]
                dg = wk.tile([P, ncols], f32, tag="dg")
                nc.sync.dma_start(
                    dg[:], dd_in.ap().rearrange("(p c) -> p c", p=P)
                    [:, col0:col0 + ncols])
                nc.vector.reciprocal(dg[:], dg[:])
                nc.scalar.sqrt(dg[:], dg[:])
                nc.vector.tensor_tensor(
                    out=v[:], in0=v[:],
                    in1=dg[:][:, :, None].to_broadcast([P, ncols, 2]),
                    op=mybir.AluOpType.mult)
                # graph ids for these cols
                gid = wk.tile([P, ncols], f32, tag="gid")
                nc.sync.dma_start(
                    gid[:], g_in.ap().rearrange("(p c) -> p c", p=P)
                    [:, col0:col0 + ncols])
                # one-hot per super of SC cols, then per-col matmul
                SC = 64
                for s0 in range(0, ncols, SC):
                    sc = min(SC, ncols - s0)
                    oh = wk.tile([P, SC, GPC], f32, tag="oh")
                    gslice = gid[:, s0:s0 + sc]
                    g_b = bass.AP(gslice.tensor, gslice.offset,
                                  [gslice.ap[0], gslice.ap[1], [0, GPC]])
                    i_b = bass.AP(iota[:].tensor, iota[:].offset,
                                  [iota[:].ap[0], [0, sc], iota[:].ap[1]])
                    nc.vector.tensor_tensor(out=oh[:, :sc, :], in0=g_b,
                                            in1=i_b,
                                            op=mybir.AluOpType.is_equal)
                    for c in range(sc):
                        done_cols += 1
                        nc.tensor.matmul(
                            acc[:], lhsT=v[:, s0 + c, :], rhs=oh[:, c, :],
                            start=False, stop=(done_cols == total_cols))
                nodecol += rows

            accs = cpool.tile([2, GPC], f32, tag="accs")
            nc.vector.tensor_copy(accs[:], acc[:])

            # scale by 1/cnt, add b2' = b2 @ Wl + bl
            cnts = cpool.tile([1, GPC], f32, tag="cnts")
            nc.sync.dma_start(cnts[:], cnt_in.ap()[None, :])
            rec = cpool.tile([1, GPC], f32, tag="rec")
            nc.vector.reciprocal(rec[:], cnts[:])
            rec2 = cpool.tile([2, GPC], f32, tag="rec2")
            nc.sync.dma_start(rec2[0:1, :], rec[:])
            nc.sync.dma_start(rec2[1:2, :], rec[:])
            nc.vector.tensor_tensor(out=accs[:], in0=accs[:], in1=rec2[:],
                                    op=mybir.AluOpType.mult)

            W2T = cpool.tile([2, 16], f32, tag="w2t")
            nc.sync.dma_start(W2T[:], W2_in.ap().rearrange("a b -> b a"))
            Wlt = cpool.tile([2, 2], f32, tag="wl")
            nc.sync.dma_start(Wlt[:], Wl_in.ap())
            b2t = cpool.tile([2, 1], f32, tag="b2t")
            nc.sync.dma_start(b2t[:], b2_in.ap()[:, None])
            b2w_ps = ps.tile([1, 2], f32, tag="b2w")
            nc.tensor.matmul(b2w_ps[:], lhsT=b2t[:], rhs=Wlt[:],
                             start=True, stop=True)
            blt = cpool.tile([1, 2], f32, tag="blt")
            nc.sync.dma_start(blt[:], bl_in.ap()[None, :])
            b2p = cpool.tile([1, 2], f32, tag="b2p")
            nc.vector.tensor_add(b2p[:], b2w_ps[:], blt[:])
            # move [1,2] -> [2,1] (two tiny copies to different partitions)
            b2pt = cpool.tile([2, 1], f32, tag="b2pt")
            nc.sync.dma_start(b2pt[0:1, 0:1], b2p[0:1, 0:1])
            nc.sync.dma_start(b2pt[1:2, 0:1], b2p[0:1, 1:2])
            nc.vector.tensor_scalar_add(accs[:], accs[:], b2pt[:, 0:1])

            nc.sync.dma_start(out.ap(), accs[:])
    nc.compile()
    return nc


# ---------------------------------------------------------------- entry point

def kernel(x, edge_index, batch, W1, b1, W2, b2, Wl, bl):
    x = np.asarray(x, np.float32)
    edge_index = np.asarray(edge_index)
    batch = np.asarray(batch)
    N = x.shape[0]
    src = edge_index[0].astype(np.int64)
    dst = edge_index[1].astype(np.int64)

    deg = (np.bincount(dst, minlength=N) + 1).astype(np.float32)  # incl self

    gb = np.searchsorted(batch, np.arange(NUM_GRAPHS + 1))
    nbounds = [int(gb[GPC * c]) for c in range(NC)] + [N]

    # per-core edge sets
    cores = []
    for c in range(NC):
        n0, n1 = nbounds[c], nbounds[c + 1]
        mask = (dst >= n0) & (dst < n1)
        cores.append((n0, n1 - n0, src[mask], (dst[mask] - n0)))

    # common L1 bins: width W = deg (int), counts = max over cores
    per_core_cnt = []
    for (n0, V_c, s, d) in cores:
        cc = Counter(deg[n0:n0 + V_c].astype(np.int64).tolist())
        per_core_cnt.append(cc)
    common_bins = {}
    for cc in per_core_cnt:
        for w, n in cc.items():
            common_bins[w] = max(common_bins.get(w, 0), n)

    l1_stage = []
    for c, (n0, V_c, s, d) in enumerate(cores):
        l1_stage.append(_build_core_l1(c, x, s, d, deg, n0, V_c, common_bins))
    bins, COLS, NR = l1_stage[0][3], l1_stage[0][4], l1_stage[0][5]

    key1 = ("l1", tuple((w, c) for (w, c, _, _) in bins))
    if key1 not in _PROG_CACHE:
        _PROG_CACHE[key1] = _build_l1_program(bins, COLS, NR)
    nc1 = _PROG_CACHE[key1]
    in_maps1 = []
    for c in range(NC):
        x_slot, deg_slot = l1_stage[c][0], l1_stage[c][1]
        in_maps1.append({"x_slot": x_slot, "deg_slot": deg_slot,
                         "W1": np.asarray(W1, np.float32),
                         "b1": np.asarray(b1, np.float32),
                         "W2": np.asarray(W2, np.float32),
                         "Wl": np.asarray(Wl, np.float32)})
    import time as _time
    from concurrent.futures import ThreadPoolExecutor

    def _run1():
        try:
            return run_bass_kernel_spmd(nc1, in_maps1,
                                        core_ids=list(range(NC)), trace=TRACE)
        except ModuleNotFoundError:
            return run_bass_kernel_spmd(nc1, in_maps1,
                                        core_ids=list(range(NC)))

    _t = _time.time()
    _ex = ThreadPoolExecutor(1)
    _fut1 = _ex.submit(_run1)

    # L2 common bins
    per_core_w2 = []
    for (n0, V_c, s, d) in cores:
        w2 = np.bincount(s, minlength=N)
        w2[n0:n0 + V_c] += 1
        per_core_w2.append(w2)
    common_bins2 = {}
    for w2 in per_core_w2:
        cc = Counter(w2[w2 > 0].tolist())
        for w, n in cc.items():
            common_bins2[w] = max(common_bins2.get(w, 0), n)

    l2_stage = []
    for c, (n0, V_c, s, d) in enumerate(cores):
        batch_l = (batch[n0:n0 + V_c] - GPC * c).astype(np.int64)
        l2_stage.append(_build_core_l2(c, s, d, deg, batch_l,
                                       n0, V_c, common_bins2, N))
    bins2, COLS2, NR2 = l2_stage[0][3], l2_stage[0][4], l2_stage[0][5]

    res1 = _fut1.result()
    _ex.shutdown()
    LAST_WALL_S.append(_time.time() - _t)
    LAST_RESULTS.append(res1)
    LAST_EXEC_NS.append(res1.exec_time_ns)

    # host: assemble m_global, then fill m_src per core (cheap fancy-index)
    m_global = np.zeros((N, 2), np.float32)
    for c, (n0, V_c, s, d) in enumerate(cores):
        node_map = l1_stage[c][2]
        m_flat = res1.results[c]["m"].reshape(P, NR, 2)
        valid = node_map >= 0
        m_global[n0 + node_map[valid]] = m_flat[valid]

    key2 = ("l2", tuple((w, c) for (w, c, _, _) in bins2))
    if key2 not in _PROG_CACHE:
        _PROG_CACHE[key2] = _build_l2_program(bins2, COLS2, NR2)
    nc2 = _PROG_CACHE[key2]
    iota32 = np.broadcast_to(np.arange(GPC, dtype=np.float32)[None, :],
                             (P, GPC)).copy()
    in_maps2 = []
    for c in range(NC):
        node_grid, graphid, degdst = (l2_stage[c][0], l2_stage[c][1],
                                      l2_stage[c][2])
        m_src = np.zeros((P * NR2, 2), np.float32)
        gv = node_grid >= 0
        m_src[gv] = m_global[node_grid[gv]]
        n0, V_c = cores[c][0], cores[c][1]
        cnts = np.maximum(
            np.bincount(batch[n0:n0 + V_c].astype(np.int64) - GPC * c,
                        minlength=GPC), 1).astype(np.float32)
        in_maps2.append({"m_src": m_src, "graphid": graphid, "degdst": degdst,
                         "cnts": cnts, "iota32": iota32,
                         "W2": np.asarray(W2, np.float32),
                         "Wl": np.asarray(Wl, np.float32),
                         "b2": np.asarray(b2, np.float32),
                         "bl": np.asarray(bl, np.float32)})
    _t = _time.time()
    try:
        res2 = run_bass_kernel_spmd(nc2, in_maps2, core_ids=list(range(NC)),
                                    trace=TRACE)
    except ModuleNotFoundError:
        res2 = run_bass_kernel_spmd(nc2, in_maps2, core_ids=list(range(NC)))
    LAST_WALL_S.append(_time.time() - _t)
    LAST_RESULTS.append(res2)
    LAST_EXEC_NS.append(res2.exec_time_ns)

    out = np.zeros((NUM_GRAPHS, 2), np.float32)
    for c in range(NC):
        o = res2.results[c]["out"]  # [2, GPC]
        out[GPC * c:GPC * (c + 1)] = o.T
    # empty graphs -> 0 (reference: sums/max(cnt,1) with sums=0)
    sizes = np.diff(gb)
    out[sizes == 0] = 0.0
    return out
